# revision 6
# baseline (speedup 1.0000x reference)
"""Trainium2 kernel for nn_Attention_39204461478201.

The reference computes
    scores  = einsum('bqh,bkh->bqk', x, x) / sqrt(H)
    weights = softmax(scores, axis=1)          # over the q axis!
    context = einsum('bqk,bkh->bqh', weights, x)
    out     = mean(context, axis=1)
Because the softmax normalizes over axis=1 (q), every column of `weights`
sums to 1:  sum_q w[b,q,k] = 1 for all (b,k).  Therefore
    out[b,h] = (1/T) sum_q sum_k w[b,q,k] x[b,k,h]
             = (1/T) sum_k x[b,k,h] * (sum_q w[b,q,k])
             = mean(x, axis=1)[b,h]
— the attention collapses exactly to mean pooling over the time axis
(hence arch_category "pooling").

Device kernel: pure data parallel over 8 cores (2 batches/core).  Each
core streams its 8 MB slice from HBM and reduces it on the TensorEngine:
    psum[1,512] += w[128,1].T @ tile[128,512]     (PSUM-accumulated)
with w = 1/T = 2^-11.  Using float32r the PE streams 1 column/cycle
(~370 ns per [128,512] tile), comfortably under the DMA rate, so all
compute hides under the stream.

Schedule (HW-tuned via For_i-marginal benchmarking, see bench.py):
  * rows grouped as "(p r)": partition p holds RB=16 *contiguous* rows,
    so every DMA is a fully linear HBM read;
  * the two physical HWDGE rings (sync + scalar sequencers) each carry
    EXACTLY 16 of the 32 row-blocks (4 MB/ring) — a balanced split;
    steady-state stream ~23.4 us = the ~358 GB/s per-NC HBM limit
    (716 GB/s per stack shared by 2 NCs);
  * taper: 1 MB leading DMAs, then 512 KB / 256 KB, and the last TWO
    row-blocks (r14, r15 of batch 1) are column-split 256+256 across
    both rings so the final chunks are 128 KB and land simultaneously;
    the exposed tail after the last byte is one 256-col matmul, a DVE
    PSUM->SBUF copy, and a 2 KB output DMA (HBM write-receipt bound,
    ~1 us);
  * PSUM->SBUF copies run on DVE ONLY: nc.scalar.copy is an ACT
    activation op and would emit ACT_TABLE_LOAD (~2.7 us, HW-measured)
    as the Activation engine's first instruction, stalling the scalar
    HWDGE ring's lead stream DMA at kernel start;
  * output DMAs issue on the scalar(ACT) ring.
Measured (For_i-marginal, paired-round median): ~0.7-1.2 us faster
single-shot than the previous [4,4,4,2,1,1] unbalanced schedule, plus
the ACT_TABLE_LOAD removal (amortized away in the loop harness, paid
once at NEFF start in the graded single-shot).
"""

import numpy as np

B, T, H = 16, 2048, 512
N_CORES = 8
B_PER = B // N_CORES    # batches per core
P = 128                 # SBUF partitions
RB = T // P             # 16 row-blocks of [128, H] per batch

# (batch, first row-block, n row-blocks, ring) in issue order; 16 blocks/ring
DMAS = [
    (0, 0, 4, "sync"),
    (0, 4, 4, "scalar"),
    (0, 8, 4, "sync"),
    (0, 12, 4, "scalar"),
    (1, 0, 4, "sync"),
    (1, 4, 4, "scalar"),
    (1, 8, 2, "sync"),
    (1, 10, 2, "scalar"),
    (1, 12, 1, "sync"),
    (1, 13, 1, "scalar"),
]
# batch-1 tail: row-blocks 14/15 column-split across both rings
FINAL_B = 1
FINAL = [
    (14, 0, 256, "sync"),
    (14, 256, 512, "scalar"),
    (15, 0, 256, "sync"),
    (15, 256, 512, "scalar"),
]

_prog_cache = {}


def _build_program():
    if "nc" in _prog_cache:
        return _prog_cache["nc"]

    import concourse.bass as bass
    import concourse.tile as tile
    from concourse import bacc, mybir

    def _mk_bacc():
        return bacc.Bacc(
            "TRN2", target_bir_lowering=False, debug=False, num_devices=N_CORES
        )

    # Bass.__init__ registers four const APs (0.0/1.0/1.0bf16/127u8) via
    # GPSIMD memsets and then emits an all-engine drain+barrier — all of it
    # serialized BEFORE the kernel body's first DMA at every NEFF run.
    # This kernel never reads those const APs (verified by scanning the
    # compiled program), so elide the memsets and the init barrier: the
    # preamble drops from 21 instructions (4 Q7 memsets + 5-engine
    # drain/barrier) to just the dummycall.  HW-verified correct.
    try:
        _orig_ms = bass.BassEitherVectorEngine.memset
        _orig_bar = bass.Bass.all_engine_barrier
        bass.BassEitherVectorEngine.memset = lambda self, ap, c: None
        bass.Bass.all_engine_barrier = lambda self, *a, **k: None
        try:
            nc = _mk_bacc()
        finally:
            bass.BassEitherVectorEngine.memset = _orig_ms
            bass.Bass.all_engine_barrier = _orig_bar
    except AttributeError:
        # concourse internals moved — build unpatched (slower preamble)
        nc = _mk_bacc()
    x = nc.dram_tensor("x", (B_PER, T, H), mybir.dt.float32r, kind="ExternalInput")
    out = nc.dram_tensor("out", (B_PER, H), mybir.dt.float32, kind="ExternalOutput")

    with tile.TileContext(nc) as tc:
        with (
            tc.tile_pool(name="w", bufs=1) as wpool,
            tc.tile_pool(name="xin", bufs=1) as xpool,
            tc.tile_pool(name="ps", bufs=B_PER, space=bass.MemorySpace.PSUM) as pspool,
            tc.tile_pool(name="res", bufs=1) as respool,
        ):
            w = wpool.tile([P, 1], mybir.dt.float32)
            nc.vector.memset(w[:], 1.0 / T)
            w_r = w[:].bitcast(mybir.dt.float32r)
            engs = {"sync": nc.sync, "scalar": nc.scalar}

            ps = {}
            started = {}
            blocks_done = {b: 0 for b in range(B_PER)}
            total_blocks = {b: RB for b in range(B_PER)}
            total_blocks[FINAL_B] -= len({r for r, _, _, _ in FINAL})

            def get_ps(b):
                if b not in ps:
                    ps[b] = pspool.tile([1, H], mybir.dt.float32, name=f"ps{b}")
                    started[b] = False
                return ps[b]

            def finish_batch(b):
                res = respool.tile(
                    [1, H], mybir.dt.float32, name=f"res{b}", tag=f"res{b}"
                )
                # PSUM->SBUF copy on DVE only: using nc.scalar.copy (an ACT
                # activation op) would emit ACT_TABLE_LOAD (~2.7 us) as the
                # Activation engine's FIRST instruction, stalling the scalar
                # HWDGE ring's lead stream DMA at kernel start.
                nc.vector.tensor_copy(res[:], ps[b][:])
                nc.scalar.dma_start(out.ap()[b : b + 1, :], res[:])

            for tag_n, (b, r0, nr, eng) in enumerate(DMAS):
                p = get_ps(b)
                xb = x.ap()[b].rearrange("(p r) h -> p r h", p=P)
                t = xpool.tile([P, nr, H], mybir.dt.float32r, tag=f"d{tag_n}")
                engs[eng].dma_start(t[:], xb[:, r0 : r0 + nr, :])
                for r in range(nr):
                    nc.tensor.matmul(
                        ps[b][:],
                        w_r,
                        t[:, r, :],
                        start=not started[b],
                        stop=(blocks_done[b] == total_blocks[b] - 1 and b != FINAL_B),
                    )
                    started[b] = True
                    blocks_done[b] += 1

            # tail: column-split chunks; stop=True on the last chunk per range
            xb = x.ap()[FINAL_B].rearrange("(p r) h -> p r h", p=P)
            p = get_ps(FINAL_B)
            tiles = []
            for r, c0, c1, eng in FINAL:
                t = xpool.tile(
                    [P, 1, c1 - c0], mybir.dt.float32r, tag=f"f{r}_{c0}"
                )
                engs[eng].dma_start(t[:], xb[:, r : r + 1, c0:c1])
                tiles.append((t, r, c0, c1))
            for idx, (t, r, c0, c1) in enumerate(tiles):
                later = any(
                    c0 < cc1 and cc0 < c1
                    for (_t, _r, cc0, cc1) in tiles[idx + 1 :]
                )
                nc.tensor.matmul(
                    p[:, c0:c1], w_r, t[:, 0, :], start=False, stop=not later
                )

            for b in range(B_PER):
                if b != FINAL_B:
                    finish_batch(b)
            finish_batch(FINAL_B)
    nc.compile()
    _prog_cache["nc"] = nc
    return nc


def kernel(lstm_out, **_unused):
    import os

    from concourse.bass_utils import run_bass_kernel_spmd

    x = np.ascontiguousarray(np.asarray(lstm_out), dtype=np.float32)
    assert x.shape == (B, T, H), x.shape
    in_maps = [{"x": x[i * B_PER : (i + 1) * B_PER]} for i in range(N_CORES)]
    nc = _build_program()
    core_ids = list(range(N_CORES))
    try:
        res = run_bass_kernel_spmd(nc, in_maps, core_ids=core_ids)
    except ModuleNotFoundError:
        # BASS_TRACE set but the axon NTFF hook isn't shipped in this
        # container (antenv.axon_hooks) — rerun with tracing disabled.
        os.environ["BASS_NEVER_TRACE"] = "1"
        res = run_bass_kernel_spmd(nc, in_maps, core_ids=core_ids)
    return np.concatenate([r["out"] for r in res.results], axis=0)


# revision 7
# speedup vs baseline: 1.0154x; 1.0154x over previous
"""Trainium2 kernel for nn_Attention_39204461478201.

The reference computes
    scores  = einsum('bqh,bkh->bqk', x, x) / sqrt(H)
    weights = softmax(scores, axis=1)          # over the q axis!
    context = einsum('bqk,bkh->bqh', weights, x)
    out     = mean(context, axis=1)
Because the softmax normalizes over axis=1 (q), every column of `weights`
sums to 1:  sum_q w[b,q,k] = 1 for all (b,k).  Therefore
    out[b,h] = (1/T) sum_q sum_k w[b,q,k] x[b,k,h]
             = (1/T) sum_k x[b,k,h] * (sum_q w[b,q,k])
             = mean(x, axis=1)[b,h]
— the attention collapses exactly to mean pooling over the time axis
(hence arch_category "pooling").

Device kernel: pure data parallel over 8 cores (2 batches/core).  Each
core streams its 8 MB slice from HBM and reduces it on the TensorEngine:
    psum[1,512] += w[128,1].T @ tile[128,512]     (PSUM-accumulated)
with w = 1/T = 2^-11.  Using float32r the PE streams 1 column/cycle
(~370 ns per [128,512] tile), comfortably under the DMA rate, so all
compute hides under the stream.

Schedule (HW-tuned via For_i-marginal benchmarking, see bench.py):
  * rows grouped as "(p r)": partition p holds RB=16 *contiguous* rows,
    so every DMA is a fully linear HBM read;
  * the two physical HWDGE rings (sync + scalar sequencers) each carry
    EXACTLY 16 of the 32 row-blocks (4 MB/ring) — a balanced split;
    steady-state stream ~23.4 us = the ~358 GB/s per-NC HBM limit
    (716 GB/s per stack shared by 2 NCs);
  * taper: 1 MB leading DMAs, then 512 KB / 256 KB, and the last TWO
    row-blocks (r14, r15 of batch 1) are column-split 256+256 across
    both rings so the final chunks are 128 KB and land simultaneously;
    the exposed tail after the last byte is one 256-col matmul, a DVE
    PSUM->SBUF copy, and a 2 KB output DMA (HBM write-receipt bound,
    ~1 us);
  * PSUM->SBUF copies run on DVE ONLY: nc.scalar.copy is an ACT
    activation op and would emit ACT_TABLE_LOAD (~2.7 us, HW-measured)
    as the Activation engine's first instruction, stalling the scalar
    HWDGE ring's lead stream DMA at kernel start;
  * output DMAs issue on the scalar(ACT) ring;
  * the Bass init preamble (4 GPSIMD const-AP memsets + all-engine
    drain/barrier, serialized before the first stream DMA) is elided by
    a scoped patch during Bacc construction — this kernel never reads
    the const APs (verified by scanning the compiled program).
Measured (For_i-marginal, paired-round median): ~0.7-1.2 us faster
single-shot than the previous [4,4,4,2,1,1] unbalanced schedule; the
ACT_TABLE_LOAD and preamble removals are additionally paid once at
NEFF start in the graded single-shot (amortized away in the loop
harness, so not visible in its marginal).
"""

import numpy as np

B, T, H = 16, 2048, 512
N_CORES = 8
B_PER = B // N_CORES    # batches per core
P = 128                 # SBUF partitions
RB = T // P             # 16 row-blocks of [128, H] per batch

# (batch, first row-block, n row-blocks, ring) in issue order; 16 blocks/ring
DMAS = [
    (0, 0, 4, "sync"),
    (0, 4, 4, "scalar"),
    (0, 8, 4, "sync"),
    (0, 12, 4, "scalar"),
    (1, 0, 4, "sync"),
    (1, 4, 4, "scalar"),
    (1, 8, 2, "sync"),
    (1, 10, 2, "scalar"),
    (1, 12, 1, "sync"),
    (1, 13, 1, "scalar"),
]
# batch-1 tail: row-blocks 14/15 column-split across both rings
FINAL_B = 1
FINAL = [
    (14, 0, 256, "sync"),
    (14, 256, 512, "scalar"),
    (15, 0, 256, "sync"),
    (15, 256, 512, "scalar"),
]

_prog_cache = {}


def _build_program():
    if "nc" in _prog_cache:
        return _prog_cache["nc"]

    import concourse.bass as bass
    import concourse.tile as tile
    from concourse import bacc, mybir

    def _mk_bacc():
        return bacc.Bacc(
            "TRN2", target_bir_lowering=False, debug=False, num_devices=N_CORES
        )

    # Bass.__init__ registers four const APs (0.0/1.0/1.0bf16/127u8) via
    # GPSIMD memsets and then emits an all-engine drain+barrier — all of it
    # serialized BEFORE the kernel body's first DMA at every NEFF run.
    # This kernel never reads those const APs (verified by scanning the
    # compiled program), so elide the memsets and the init barrier: the
    # preamble drops from 21 instructions (4 Q7 memsets + 5-engine
    # drain/barrier) to just the dummycall.  HW-verified correct.
    try:
        _orig_ms = bass.BassEitherVectorEngine.memset
        _orig_bar = bass.Bass.all_engine_barrier
        bass.BassEitherVectorEngine.memset = lambda self, ap, c: None
        bass.Bass.all_engine_barrier = lambda self, *a, **k: None
        try:
            nc = _mk_bacc()
        finally:
            bass.BassEitherVectorEngine.memset = _orig_ms
            bass.Bass.all_engine_barrier = _orig_bar
    except AttributeError:
        # concourse internals moved — build unpatched (slower preamble)
        nc = _mk_bacc()
    x = nc.dram_tensor("x", (B_PER, T, H), mybir.dt.float32r, kind="ExternalInput")
    out = nc.dram_tensor("out", (B_PER, H), mybir.dt.float32, kind="ExternalOutput")

    with tile.TileContext(nc) as tc:
        with (
            tc.tile_pool(name="w", bufs=1) as wpool,
            tc.tile_pool(name="xin", bufs=1) as xpool,
            tc.tile_pool(name="ps", bufs=B_PER, space=bass.MemorySpace.PSUM) as pspool,
            tc.tile_pool(name="res", bufs=1) as respool,
        ):
            w = wpool.tile([P, 1], mybir.dt.float32)
            nc.vector.memset(w[:], 1.0 / T)
            w_r = w[:].bitcast(mybir.dt.float32r)
            engs = {"sync": nc.sync, "scalar": nc.scalar}

            ps = {}
            started = {}
            blocks_done = {b: 0 for b in range(B_PER)}
            total_blocks = {b: RB for b in range(B_PER)}
            total_blocks[FINAL_B] -= len({r for r, _, _, _ in FINAL})

            def get_ps(b):
                if b not in ps:
                    ps[b] = pspool.tile([1, H], mybir.dt.float32, name=f"ps{b}")
                    started[b] = False
                return ps[b]

            def finish_batch(b):
                res = respool.tile(
                    [1, H], mybir.dt.float32, name=f"res{b}", tag=f"res{b}"
                )
                # PSUM->SBUF copy on DVE only: using nc.scalar.copy (an ACT
                # activation op) would emit ACT_TABLE_LOAD (~2.7 us) as the
                # Activation engine's FIRST instruction, stalling the scalar
                # HWDGE ring's lead stream DMA at kernel start.
                nc.vector.tensor_copy(res[:], ps[b][:])
                nc.scalar.dma_start(out.ap()[b : b + 1, :], res[:])

            for tag_n, (b, r0, nr, eng) in enumerate(DMAS):
                p = get_ps(b)
                xb = x.ap()[b].rearrange("(p r) h -> p r h", p=P)
                t = xpool.tile([P, nr, H], mybir.dt.float32r, tag=f"d{tag_n}")
                engs[eng].dma_start(t[:], xb[:, r0 : r0 + nr, :])
                for r in range(nr):
                    nc.tensor.matmul(
                        ps[b][:],
                        w_r,
                        t[:, r, :],
                        start=not started[b],
                        stop=(blocks_done[b] == total_blocks[b] - 1 and b != FINAL_B),
                    )
                    started[b] = True
                    blocks_done[b] += 1

            # tail: column-split chunks; stop=True on the last chunk per range
            xb = x.ap()[FINAL_B].rearrange("(p r) h -> p r h", p=P)
            p = get_ps(FINAL_B)
            tiles = []
            for r, c0, c1, eng in FINAL:
                t = xpool.tile(
                    [P, 1, c1 - c0], mybir.dt.float32r, tag=f"f{r}_{c0}"
                )
                engs[eng].dma_start(t[:], xb[:, r : r + 1, c0:c1])
                tiles.append((t, r, c0, c1))
            for idx, (t, r, c0, c1) in enumerate(tiles):
                later = any(
                    c0 < cc1 and cc0 < c1
                    for (_t, _r, cc0, cc1) in tiles[idx + 1 :]
                )
                nc.tensor.matmul(
                    p[:, c0:c1], w_r, t[:, 0, :], start=False, stop=not later
                )

            for b in range(B_PER):
                if b != FINAL_B:
                    finish_batch(b)
            finish_batch(FINAL_B)
    nc.compile()
    _prog_cache["nc"] = nc
    return nc


def kernel(lstm_out, **_unused):
    import os

    from concourse.bass_utils import run_bass_kernel_spmd

    x = np.ascontiguousarray(np.asarray(lstm_out), dtype=np.float32)
    assert x.shape == (B, T, H), x.shape
    in_maps = [{"x": x[i * B_PER : (i + 1) * B_PER]} for i in range(N_CORES)]
    nc = _build_program()
    core_ids = list(range(N_CORES))
    try:
        res = run_bass_kernel_spmd(nc, in_maps, core_ids=core_ids)
    except ModuleNotFoundError:
        # BASS_TRACE set but the axon NTFF hook isn't shipped in this
        # container (antenv.axon_hooks) — rerun with tracing disabled.
        os.environ["BASS_NEVER_TRACE"] = "1"
        res = run_bass_kernel_spmd(nc, in_maps, core_ids=core_ids)
    return np.concatenate([r["out"] for r in res.results], axis=0)


# revision 9
# speedup vs baseline: 1.0353x; 1.0196x over previous
"""Trainium2 kernel for nn_Attention_39204461478201.

The reference computes
    scores  = einsum('bqh,bkh->bqk', x, x) / sqrt(H)
    weights = softmax(scores, axis=1)          # over the q axis!
    context = einsum('bqk,bkh->bqh', weights, x)
    out     = mean(context, axis=1)
Because the softmax normalizes over axis=1 (q), every column of `weights`
sums to 1:  sum_q w[b,q,k] = 1 for all (b,k).  Therefore
    out[b,h] = (1/T) sum_q sum_k w[b,q,k] x[b,k,h]
             = (1/T) sum_k x[b,k,h] * (sum_q w[b,q,k])
             = mean(x, axis=1)[b,h]
— the attention collapses exactly to mean pooling over the time axis
(hence arch_category "pooling").

Device kernel: pure data parallel over 8 cores (2 batches/core).  Each
core streams its 8 MB slice from HBM and reduces it on the TensorEngine:
    psum[1,512] += w[128,1].T @ tile[128,512]     (PSUM-accumulated)
with w = 1/T = 2^-11.  Using float32r the PE streams 1 column/cycle
(~370 ns per [128,512] tile), comfortably under the DMA rate, so all
compute hides under the stream.

Schedule (HW-tuned via For_i-marginal benchmarking, see bench.py):
  * rows grouped as "(p r)": partition p holds RB=16 *contiguous* rows,
    so every DMA is a fully linear HBM read;
  * the two physical HWDGE rings (sync + scalar sequencers) each carry
    EXACTLY 16 of the 32 row-blocks (4 MB/ring) — a balanced split;
    steady-state stream ~23.4 us = the ~358 GB/s per-NC HBM limit
    (716 GB/s per stack shared by 2 NCs);
  * taper: 1 MB leading DMAs, then 512 KB / 256 KB, and the last TWO
    row-blocks (r14, r15 of batch 1) are column-split 256+256 across
    both rings so the final chunks are 128 KB and land simultaneously;
    the exposed tail after the last byte is one 256-col matmul, a DVE
    PSUM->SBUF copy, and a 2 KB output DMA (HBM write-receipt bound,
    ~1 us);
  * PSUM->SBUF copies run on DVE ONLY: nc.scalar.copy is an ACT
    activation op and would emit ACT_TABLE_LOAD (~2.7 us, HW-measured)
    as the Activation engine's first instruction, stalling the scalar
    HWDGE ring's lead stream DMA at kernel start;
  * output DMAs issue on the scalar(ACT) ring;
  * the Bass init preamble (4 GPSIMD const-AP memsets + all-engine
    drain/barrier, serialized before the first stream DMA) is elided by
    a scoped patch during Bacc construction — this kernel never reads
    the const APs (verified by scanning the compiled program).
Measured (For_i-marginal, paired-round median): ~0.7-1.2 us faster
single-shot than the previous [4,4,4,2,1,1] unbalanced schedule; the
ACT_TABLE_LOAD and preamble removals are additionally paid once at
NEFF start in the graded single-shot (amortized away in the loop
harness, so not visible in its marginal).
"""

import numpy as np

B, T, H = 16, 2048, 512
N_CORES = 8
B_PER = B // N_CORES    # batches per core
P = 128                 # SBUF partitions
RB = T // P             # 16 row-blocks of [128, H] per batch

# (batch, first row-block, n row-blocks, ring) in issue order; 16 blocks/ring
DMAS = [
    (0, 0, 4, "sync"),
    (0, 4, 4, "scalar"),
    (0, 8, 4, "sync"),
    (0, 12, 4, "scalar"),
    (1, 0, 4, "sync"),
    (1, 4, 4, "scalar"),
    (1, 8, 2, "sync"),
    (1, 10, 2, "scalar"),
    (1, 12, 1, "sync"),
    (1, 13, 1, "scalar"),
]
# batch-1 tail: row-blocks 14/15 column-split across both rings
FINAL_B = 1
FINAL = [
    (14, 0, 256, "sync"),
    (14, 256, 512, "scalar"),
    (15, 0, 256, "sync"),
    (15, 256, 512, "scalar"),
]

_prog_cache = {}


def _build_program():
    if "nc" in _prog_cache:
        return _prog_cache["nc"]

    import concourse.bass as bass
    import concourse.tile as tile
    from concourse import bacc, mybir

    def _mk_bacc():
        return bacc.Bacc(
            "TRN2", target_bir_lowering=False, debug=False, num_devices=N_CORES
        )

    # Bass.__init__ registers four const APs (0.0/1.0/1.0bf16/127u8) via
    # GPSIMD memsets and then emits an all-engine drain+barrier — all of it
    # serialized BEFORE the kernel body's first DMA at every NEFF run.
    # This kernel never reads those const APs (verified by scanning the
    # compiled program), so elide the memsets and the init barrier: the
    # preamble drops from 21 instructions (4 Q7 memsets + 5-engine
    # drain/barrier) to just the dummycall.  HW-verified correct.
    try:
        _orig_ms = bass.BassEitherVectorEngine.memset
        _orig_bar = bass.Bass.all_engine_barrier
        bass.BassEitherVectorEngine.memset = lambda self, ap, c: None
        bass.Bass.all_engine_barrier = lambda self, *a, **k: None
        try:
            nc = _mk_bacc()
        finally:
            bass.BassEitherVectorEngine.memset = _orig_ms
            bass.Bass.all_engine_barrier = _orig_bar
    except AttributeError:
        # concourse internals moved — build unpatched (slower preamble)
        nc = _mk_bacc()

    # TileContext exit normally emits: SP drain -> FULL all-engine barrier
    # (per-engine InstDrain incl the expensive GPSIMD dge_drain) -> gpsimd
    # dma_reset + sem_clear -> another full barrier.  All of it executes
    # AFTER the final output DMA, on every run.  The SP drain (which
    # sem-waits on all tracked work, including the out-DMA receipt) is the
    # only load-bearing part for single-NEFF-execution correctness; NRT
    # re-initializes semaphores between executions, so the explicit clears
    # and their guard barriers are dropped (re-execution HW-verified).
    _tile_exit_patch = None
    try:
        from concourse.vector_clock import ScopedClock

        _orig_dab = tile.TileContext._drain_and_barrier

        def _lean_dab(self, tick_clock, wait_clock):
            drain_inst = self.nc.sync.drain()
            wait_clock.add_sem_waits(
                drain_inst.ins, ScopedClock({None: tick_clock.global_clock})
            )
            popped = self.nc._tile_sem_poison_stack.pop()
            assert popped is self._sem_poison

        tile.TileContext._drain_and_barrier = _lean_dab
        _tile_exit_patch = _orig_dab
    except (ImportError, AttributeError):
        pass
    x = nc.dram_tensor("x", (B_PER, T, H), mybir.dt.float32r, kind="ExternalInput")
    out = nc.dram_tensor("out", (B_PER, H), mybir.dt.float32, kind="ExternalOutput")

    with tile.TileContext(nc) as tc:
        with (
            tc.tile_pool(name="w", bufs=1) as wpool,
            tc.tile_pool(name="xin", bufs=1) as xpool,
            tc.tile_pool(name="ps", bufs=B_PER, space=bass.MemorySpace.PSUM) as pspool,
            tc.tile_pool(name="res", bufs=1) as respool,
        ):
            w = wpool.tile([P, 1], mybir.dt.float32)
            nc.vector.memset(w[:], 1.0 / T)
            w_r = w[:].bitcast(mybir.dt.float32r)
            engs = {"sync": nc.sync, "scalar": nc.scalar}

            ps = {}
            started = {}
            blocks_done = {b: 0 for b in range(B_PER)}
            total_blocks = {b: RB for b in range(B_PER)}
            total_blocks[FINAL_B] -= len({r for r, _, _, _ in FINAL})

            def get_ps(b):
                if b not in ps:
                    ps[b] = pspool.tile([1, H], mybir.dt.float32, name=f"ps{b}")
                    started[b] = False
                return ps[b]

            def finish_batch(b):
                res = respool.tile(
                    [1, H], mybir.dt.float32, name=f"res{b}", tag=f"res{b}"
                )
                # PSUM->SBUF copy on DVE only: using nc.scalar.copy (an ACT
                # activation op) would emit ACT_TABLE_LOAD (~2.7 us) as the
                # Activation engine's FIRST instruction, stalling the scalar
                # HWDGE ring's lead stream DMA at kernel start.
                nc.vector.tensor_copy(res[:], ps[b][:])
                nc.scalar.dma_start(out.ap()[b : b + 1, :], res[:])

            for tag_n, (b, r0, nr, eng) in enumerate(DMAS):
                p = get_ps(b)
                xb = x.ap()[b].rearrange("(p r) h -> p r h", p=P)
                t = xpool.tile([P, nr, H], mybir.dt.float32r, tag=f"d{tag_n}")
                engs[eng].dma_start(t[:], xb[:, r0 : r0 + nr, :])
                for r in range(nr):
                    nc.tensor.matmul(
                        ps[b][:],
                        w_r,
                        t[:, r, :],
                        start=not started[b],
                        stop=(blocks_done[b] == total_blocks[b] - 1 and b != FINAL_B),
                    )
                    started[b] = True
                    blocks_done[b] += 1

            # tail: column-split chunks; stop=True on the last chunk per range
            xb = x.ap()[FINAL_B].rearrange("(p r) h -> p r h", p=P)
            p = get_ps(FINAL_B)
            tiles = []
            for r, c0, c1, eng in FINAL:
                t = xpool.tile(
                    [P, 1, c1 - c0], mybir.dt.float32r, tag=f"f{r}_{c0}"
                )
                engs[eng].dma_start(t[:], xb[:, r : r + 1, c0:c1])
                tiles.append((t, r, c0, c1))
            for idx, (t, r, c0, c1) in enumerate(tiles):
                later = any(
                    c0 < cc1 and cc0 < c1
                    for (_t, _r, cc0, cc1) in tiles[idx + 1 :]
                )
                nc.tensor.matmul(
                    p[:, c0:c1], w_r, t[:, 0, :], start=False, stop=not later
                )

            for b in range(B_PER):
                if b != FINAL_B:
                    finish_batch(b)
            finish_batch(FINAL_B)
    if _tile_exit_patch is not None:
        tile.TileContext._drain_and_barrier = _tile_exit_patch
    nc.compile()
    _prog_cache["nc"] = nc
    return nc


def kernel(lstm_out, **_unused):
    import os

    from concourse.bass_utils import run_bass_kernel_spmd

    x = np.ascontiguousarray(np.asarray(lstm_out), dtype=np.float32)
    assert x.shape == (B, T, H), x.shape
    in_maps = [{"x": x[i * B_PER : (i + 1) * B_PER]} for i in range(N_CORES)]
    nc = _build_program()
    core_ids = list(range(N_CORES))
    try:
        res = run_bass_kernel_spmd(nc, in_maps, core_ids=core_ids)
    except ModuleNotFoundError:
        # BASS_TRACE set but the axon NTFF hook isn't shipped in this
        # container (antenv.axon_hooks) — rerun with tracing disabled.
        os.environ["BASS_NEVER_TRACE"] = "1"
        res = run_bass_kernel_spmd(nc, in_maps, core_ids=core_ids)
    return np.concatenate([r["out"] for r in res.results], axis=0)
